# revision 1
# baseline (speedup 1.0000x reference)
"""Trainium2 Bass kernel for ChunkCausalDepthwiseConv1d.

Problem: x (16, 512, 4096) f32; per-channel depthwise convs:
  out = chunk_scale * (chunkconv_K31_same_per_256chunk(x) + chunk_b)
        + causalconv_K16(x) + causal_b

Strategy (8 NeuronCores, channel-parallel, 64 ch/core, all batches):
  The conv is cast as per-channel Toeplitz matmuls (float32r, full rate at
  N>=256) on the TensorEngine in a time-major (transposed) domain.  Host
  precomputes, per channel, 5 stationary operands with scale/bias folded in:
    A_e/A_o [128,128]: within-block taps -> even/odd 128-block of each chunk
    B [32,128]: next-block chunk-conv lookahead -> even block
    C_e/C_o [128,32]: prev-block causal / chunk+causal carry (zero-padded
      rows so no tile_position is needed; fp32r + col-offset tile_position
      miscompiles, and (96,0) corners adjacent to K=32 matmuls hang the PE)
  Per-channel device pipeline:
    1 DMA  x [128=(b16,q8), 512t] (2 KiB runs)
    4 PE transposes (fp32) -> xtm[t, j*144+b*9+q9] (f32r), zero cols at q9=0
      feed each batch's causal carry via a negative-stride rhs AP
    5 matmuls -> psum_e/psum_o [128, 256] with columns ordered (u%2, b, u//2)
      so the output transposes come out (b, q)-major
    2 ACT Identity+bias evacuations, 4 PE transposes back, 1 output DMA
      [128=(b,q), 512t].
  Weights (12.8 MiB/core) stream on the SWDGE ring so x loads start at once.
  Measured ~169 us/core steady state (~89 us DMA floor for x+out),
  rel L2 err ~1.3e-4 (float32r matmul rounding).
"""

import numpy as np

B, C, T = 16, 512, 4096
NCORES = 8
NCH = C // NCORES          # 64 channels per core
NBLK = T // 128            # 32 blocks of 128 per batch
NU = T // 256              # 16 chunks per batch
PACKW = 392                # cols: A_e 128 | A_o 128 | {B rows0:32 | C_e,C_o rows96:128} 128 | biases 2 | pad


def _pack_weights(causal_w, causal_b, chunk_w, chunk_b, conv_scale):
    """Build (C, 128, PACKW) f32 stationary operands, scale/bias folded in."""
    w1 = np.asarray(causal_w, np.float32)[:, 0, :]     # (C,16)
    b1 = np.asarray(causal_b, np.float32)              # (C,)
    w2 = np.asarray(chunk_w, np.float32)[:, 0, :]      # (C,31)
    b2 = np.asarray(chunk_b, np.float32)               # (C,)
    cs = np.asarray(conv_scale, np.float32)            # (2,C,31)

    scale = np.ones((C, 256), np.float32)
    scale[:, :31] += cs[0]
    scale[:, 225:] += cs[1]

    k = np.arange(128)[:, None]
    m = np.arange(128)[None, :]
    d = k - m + 15
    band2 = (d >= 0) & (d <= 30)
    band1 = (d >= 0) & (d <= 15)
    d2 = np.clip(d, 0, 30)
    d1 = np.clip(d, 0, 15)

    w2d = w2[:, d2] * band2            # (C,128,128)
    w1d = w1[:, d1] * band1
    A_e = w2d * scale[:, None, :128] + w1d
    A_o = w2d * scale[:, None, 128:] + w1d

    kc = np.arange(32)[:, None]
    mc = np.arange(32)[None, :]
    kB = np.arange(32)[:, None]
    mB_ = np.arange(128)[None, :]
    dB = kB + 143 - mB_                # B (next-block -> even): w2 taps, k<15, m>=113
    mB = (dB >= 0) & (dB <= 30)
    Bw = w2[:, np.clip(dB, 0, 30)] * mB            # (C,32,128)
    dC = kc - mc - 17                  # C corners: taps 0..15 / 0..14
    mC1 = (dC >= 0) & (dC <= 15)
    Ce_t = w1[:, np.clip(dC, 0, 15)] * mC1
    Co_t = (w2[:, np.clip(dC, 0, 30)] + w1[:, np.clip(dC, 0, 15)]) * mC1

    pack = np.zeros((C, 128, PACKW), np.float32)
    pack[:, :, 0:128] = A_e
    pack[:, :, 128:256] = A_o
    # B lives in rows 0:32 of cols 256:384 (nonzero only cols 369:384);
    # C_e / C_o live in rows 96:128 of cols 256:288 / 288:320 (disjoint rows).
    pack[:, 0:32, 256:384] = Bw
    pack[:, 96:128, 256:288] = Ce_t
    pack[:, 96:128, 288:320] = Co_t
    pack[:, :, 384] = scale[:, :128] * b2[:, None] + b1[:, None]   # bias_e
    pack[:, :, 385] = scale[:, 128:] * b2[:, None] + b1[:, None]   # bias_o
    return pack


def build_nc(nch=NCH, enable_asserts=False, loop_reps=1, skip=()):
    """Build the per-core Bass program (same NEFF for all cores)."""
    import concourse.bacc as bacc
    import concourse.mybir as mybir
    import concourse.tile as tile
    from concourse.ap import AP as BassAP

    fp32 = mybir.dt.float32
    fp32r = mybir.dt.float32r
    COPY = mybir.ActivationFunctionType.Identity

    nc = bacc.Bacc("TRN2", target_bir_lowering=False, debug=False,
                   enable_asserts=enable_asserts)

    x_d = nc.dram_tensor("x", [B, nch, T], fp32, kind="ExternalInput").ap()
    w_d = nc.dram_tensor("wpack", [nch, 128, PACKW], fp32r, kind="ExternalInput").ap()
    id_d = nc.dram_tensor("ident", [128, 128], fp32r, kind="ExternalInput").ap()
    o_d = nc.dram_tensor("out", [B, nch, T], fp32, kind="ExternalOutput").ap()

    # DRAM views: per channel [b16, q8, t512] <-> sbuf [128=(b,q), 512]; 2KiB runs
    x_v = x_d.rearrange("b c (q t) -> c b q t", q=8)
    o_v = o_d.rearrange("b c (q t) -> c b q t", q=8)
    # weights: chunks of channels
    WCH = min(8, nch)  # channels per weight DMA
    w_v = w_d.rearrange("(cc c) p w -> cc p c w", cc=nch // WCH)

    with tile.TileContext(nc) as tc:
        with (
            tc.tile_pool(name="wbuf", bufs=1) as wbuf_pool,
            tc.tile_pool(name="ident", bufs=1) as id_pool,
            tc.tile_pool(name="xnat", bufs=3) as xnat_pool,
            tc.tile_pool(name="xtm", bufs=2) as xtm_pool,
            tc.tile_pool(name="otm", bufs=3) as otm_pool,
            tc.tile_pool(name="onat", bufs=3) as onat_pool,
            tc.tile_pool(name="ps_it", bufs=2, space="PSUM") as psit_pool,
            tc.tile_pool(name="ps_conv", bufs=3, space="PSUM") as psconv_pool,
            tc.tile_pool(name="ps_ot", bufs=2, space="PSUM") as psot_pool,
        ):
            wbuf = wbuf_pool.tile([128, nch, PACKW], fp32r)
            ident = id_pool.tile([128, 128], fp32r)
            ztile = id_pool.tile([128, 64], fp32r, tag="ztile")
            nc.vector.memset(ztile[:].bitcast(fp32), 0.0)
            nc.gpsimd.dma_start(ident[:], id_d)
            for i in range(nch // WCH):
                nc.gpsimd.dma_start(wbuf[:, i * WCH:(i + 1) * WCH, :], w_v[i])

            import contextlib
            loop_cm = (tc.For_i(0, loop_reps, 1) if loop_reps > 1
                       else contextlib.nullcontext())
            with loop_cm:
              for c in range(nch):
                  # --- load x natural [128=(b,q), 512t], transpose ---
                  # xtm col = j*144 + b*9 + q9; block k=4q+j of batch b at
                  # q9=q+1; q9=0 cols are zero (causal carry for each batch's
                  # first chunk, via the j=3 segment).
                  xnat = xnat_pool.tile([128, 512], fp32)
                  xtm = xtm_pool.tile([128, 576], fp32r)
                  xv = xtm[:].rearrange("p (jj two b q9) -> p jj two b q9",
                                        jj=2, two=2, b=16, q9=9)
                  if "indma" not in skip:
                      nc.sync.dma_start(xnat[:], x_v[c])
                  for jj in range(2):
                      if "intrans" in skip:
                          break
                      ps = psit_pool.tile([128, 256], fp32, tag="ps_it")
                      for two in range(2):
                          j = 2 * jj + two
                          nc.tensor.transpose(
                              ps[:, two * 128:(two + 1) * 128],
                              xnat[:, j * 128:(j + 1) * 128],
                              ident[:].bitcast(fp32))
                      if "inevac" not in skip:
                          nc.vector.tensor_copy(
                              xv[:, jj, :, :, 1:9],
                              ps[:].rearrange("p (two b q) -> p two b q",
                                              two=2, b=16))
                  nc.vector.tensor_copy(xv[:, :, :, :, 0], ztile[:])

                  # --- conv matmuls (fp32r) ---
                  # psum cols ordered (up, b, uh): u = 2*uh + up.
                  rhs_even = xv[:, :, 0, :, 1:9]      # block 2u   = (jj=up, two=0)
                  rhs_odd = xv[:, :, 1, :, 1:9]       # block 2u+1 = (jj=up, two=1)
                  wA_e = wbuf[:, c, 0:128]
                  wA_o = wbuf[:, c, 128:256]
                  wB = wbuf[0:32, c, 256:384]
                  wCe = wbuf[:, c, 256:288]
                  wCo = wbuf[:, c, 288:320]

                  ps_e = psconv_pool.tile([128, 256], fp32, tag="ps_conv")
                  ps_o = psconv_pool.tile([128, 256], fp32, tag="ps_conv")
                  if "conv" not in skip:
                    nc.tensor.matmul(ps_e[:], wA_e, rhs_even,
                                     start=True, stop=False, skip_group_check=True)
                    nc.tensor.matmul(ps_e[:], wB, xv[0:32, :, 1, :, 1:9],
                                     start=False, stop=False, skip_group_check=True)
                    # causal carry rhs, cols (up, b, uh):
                    # col = 432 - 287*up + 9b + uh; up=0 hits the j=3 segment
                    # (q9=0 -> zero col), up=1 the j=1 segment.
                    rhs_prev = BassAP(tensor=xtm[:].tensor, offset=432,
                                      ap=[[576, 128], [-287, 2], [9, 16], [1, 8]])
                    nc.tensor.matmul(ps_e[0:32, :], wCe, rhs_prev,
                                     start=False, stop=True, skip_group_check=True)

                    nc.tensor.matmul(ps_o[:], wA_o, rhs_odd,
                                     start=True, stop=False, skip_group_check=True)
                    nc.tensor.matmul(ps_o[0:32, :], wCo, rhs_even,
                                     start=False, stop=True, skip_group_check=True)

                  # --- evacuate with bias ---
                  otm_e = otm_pool.tile([128, 256], fp32, tag="otm")
                  otm_o = otm_pool.tile([128, 256], fp32, tag="otm")
                  if "act" not in skip:
                      nc.scalar.activation(otm_e[:], ps_e[:], COPY,
                                           bias=wbuf[:, c, 384:385].bitcast(fp32))
                      nc.scalar.activation(otm_o[:], ps_o[:], COPY,
                                           bias=wbuf[:, c, 385:386].bitcast(fp32))

                  # --- transpose back to natural [128=(b,q), 512t], store ---
                  onat = onat_pool.tile([128, 512], fp32, tag="onat")
                  for half in range(2):
                      if "outtrans" in skip:
                          break
                      ps = psot_pool.tile([128, 256], fp32, tag="ps_ot")
                      for par, otm in ((0, otm_e), (1, otm_o)):
                          nc.tensor.transpose(
                              ps[:, par * 128:(par + 1) * 128],
                              otm[:, half * 128:(half + 1) * 128],
                              ident[:].bitcast(fp32))
                      if "outevac" not in skip:
                          nc.vector.tensor_copy(
                              onat[:, half * 256:(half + 1) * 256], ps[:])
                  if "outdma" not in skip:
                      nc.scalar.dma_start(o_v[c], onat[:])

    nc.compile()
    return nc


def kernel(x, causal_w, causal_b, chunk_w, chunk_b, conv_scale, chunk_size):
    from concourse.bass_utils import run_bass_kernel_spmd

    assert int(chunk_size) == 256
    x = np.ascontiguousarray(np.asarray(x, np.float32))
    pack = _pack_weights(causal_w, causal_b, chunk_w, chunk_b, conv_scale)
    ident = np.eye(128, dtype=np.float32)

    nc = build_nc()
    core_ids = list(range(NCORES))
    in_maps = []
    for i in core_ids:
        in_maps.append({
            "x": np.ascontiguousarray(x[:, i * NCH:(i + 1) * NCH, :]),
            "wpack": np.ascontiguousarray(pack[i * NCH:(i + 1) * NCH]),
            "ident": ident,
        })
    res = run_bass_kernel_spmd(nc, in_maps, core_ids)
    out = np.empty((B, C, T), np.float32)
    for i in core_ids:
        out[:, i * NCH:(i + 1) * NCH, :] = res.results[i]["out"]
    return out



# revision 3
# speedup vs baseline: 1.7285x; 1.7285x over previous
"""Trainium2 Bass kernel for ChunkCausalDepthwiseConv1d (bf16 pipeline).

Problem: x (16, 512, 4096) f32; per-channel depthwise convs:
  out = chunk_scale * (chunkconv_K31_same_per_256chunk(x) + chunk_b)
        + causalconv_K16(x) + causal_b

Strategy (8 NeuronCores, channel-parallel, 64 ch/core, all batches):
  Per-channel Toeplitz matmuls on the TensorEngine in a time-major
  (transposed) domain, everything in bf16 (tolerance is 2e-2 rel L2;
  bf16 end-to-end lands ~1e-3):
    - host converts x to bf16, packs per-channel stationary operands
      (A_e/A_o within-block taps with scale folded, B next-block
      lookahead, C_e/C_o prev-block carries) in bf16, biases in fp32.
    - transposes run as REGULAR matmuls (lhsT = data block, rhs =
      identity) instead of transpose-mode: they pipeline as LDW+MM
      pairs (~81ns vs ~275ns for transpose-mode) and count as PE-busy
      so the HAM clock gate stays at 2.4 GHz.
    - bf16 halves DMA (x 8 MiB + weights 6.4 MiB + out 8 MiB per core)
      and enables fast-weight-load on all 128-col stationaries.
  Per-channel device pipeline:
    1 DMA  x [128=(b16,q8), 512t] bf16 (1 KiB runs)
    4 PE matmul-transposes -> one psum bank [128, 512], 1 DVE evac ->
      xtm[t, j*144+b*9+q9] bf16, zero cols at q9=0 (memset hoisted to
      two persistent xtm buffers, alternated by channel parity)
    5 matmuls -> psum_e/psum_o [128, 256] fp32, cols (u%2, b, u//2)
    2 ACT Identity+bias (fp32 bias tensor) evacuations -> otm bf16
    4 PE matmul-transposes back, evac split DVE/ACT, 1 out DMA bf16
  Host converts the bf16 output back to fp32.
"""

import numpy as np
import ml_dtypes

B, C, T = 16, 512, 4096
NCORES = 8
NCH = C // NCORES          # 64 channels per core
NBLK = T // 128            # 32 blocks of 128 per batch
NU = T // 256              # 16 chunks per batch
PACKW = 392                # cols: A_e 128 | A_o 128 | {B rows0:32 | C_e,C_o rows96:128} 128 | pad

BF16 = ml_dtypes.bfloat16


def _pack_weights(causal_w, causal_b, chunk_w, chunk_b, conv_scale):
    """Build (C, 128, PACKW) bf16 stationary operands + (C, 128, 2) f32 biases."""
    w1 = np.asarray(causal_w, np.float32)[:, 0, :]     # (C,16)
    b1 = np.asarray(causal_b, np.float32)              # (C,)
    w2 = np.asarray(chunk_w, np.float32)[:, 0, :]      # (C,31)
    b2 = np.asarray(chunk_b, np.float32)               # (C,)
    cs = np.asarray(conv_scale, np.float32)            # (2,C,31)

    scale = np.ones((C, 256), np.float32)
    scale[:, :31] += cs[0]
    scale[:, 225:] += cs[1]

    k = np.arange(128)[:, None]
    m = np.arange(128)[None, :]
    d = k - m + 15
    band2 = (d >= 0) & (d <= 30)
    band1 = (d >= 0) & (d <= 15)
    d2 = np.clip(d, 0, 30)
    d1 = np.clip(d, 0, 15)

    w2d = w2[:, d2] * band2            # (C,128,128)
    w1d = w1[:, d1] * band1
    A_e = w2d * scale[:, None, :128] + w1d
    A_o = w2d * scale[:, None, 128:] + w1d

    kc = np.arange(32)[:, None]
    mc = np.arange(32)[None, :]
    kB = np.arange(32)[:, None]
    mB_ = np.arange(128)[None, :]
    dB = kB + 143 - mB_                # B (next-block -> even): w2 taps, k<15, m>=113
    mB = (dB >= 0) & (dB <= 30)
    Bw = w2[:, np.clip(dB, 0, 30)] * mB            # (C,32,128)
    dC = kc - mc - 17                  # C corners: taps 0..15 / 0..14
    mC1 = (dC >= 0) & (dC <= 15)
    Ce_t = w1[:, np.clip(dC, 0, 15)] * mC1
    Co_t = (w2[:, np.clip(dC, 0, 30)] + w1[:, np.clip(dC, 0, 15)]) * mC1

    pack = np.zeros((C, 128, PACKW), np.float32)
    pack[:, :, 0:128] = A_e
    pack[:, :, 128:256] = A_o
    # B lives in rows 0:32 of cols 256:384 (nonzero only cols 369:384);
    # C_e / C_o live in rows 96:128 of cols 256:288 / 288:320 (disjoint rows).
    pack[:, 0:32, 256:384] = Bw
    pack[:, 96:128, 256:288] = Ce_t
    pack[:, 96:128, 288:320] = Co_t

    bias = np.empty((C, 128, 2), np.float32)
    bias[:, :, 0] = scale[:, :128] * b2[:, None] + b1[:, None]   # bias_e
    bias[:, :, 1] = scale[:, 128:] * b2[:, None] + b1[:, None]   # bias_o
    return pack.astype(BF16), bias


def build_nc(nch=NCH, enable_asserts=False, loop_reps=1, skip=()):
    """Build the per-core Bass program (same NEFF for all cores)."""
    import concourse.bacc as bacc
    import concourse.mybir as mybir
    import concourse.tile as tile
    from concourse.ap import AP as BassAP

    fp32 = mybir.dt.float32
    bf16 = mybir.dt.bfloat16
    COPY = mybir.ActivationFunctionType.Identity

    nc = bacc.Bacc("TRN2", target_bir_lowering=False, debug=False,
                   enable_asserts=enable_asserts)

    x_d = nc.dram_tensor("x", [B, nch, T], bf16, kind="ExternalInput").ap()
    w_d = nc.dram_tensor("wpack", [nch, 128, PACKW], bf16, kind="ExternalInput").ap()
    bias_d = nc.dram_tensor("bias", [nch, 128, 2], fp32, kind="ExternalInput").ap()
    id_d = nc.dram_tensor("ident", [128, 128], bf16, kind="ExternalInput").ap()
    o_d = nc.dram_tensor("out", [B, nch, T], bf16, kind="ExternalOutput").ap()

    # DRAM views: per channel [b16, q8, t512] <-> sbuf [128=(b,q), 512]; 1KiB runs
    x_v = x_d.rearrange("b c (q t) -> c b q t", q=8)
    o_v = o_d.rearrange("b c (q t) -> c b q t", q=8)
    # weights: chunks of channels
    WCH = min(8, nch)  # channels per weight DMA
    w_v = w_d.rearrange("(cc c) p w -> cc p c w", cc=nch // WCH)
    bias_v = bias_d.rearrange("c p two -> p c two")

    with tile.TileContext(nc) as tc:
        with (
            tc.tile_pool(name="wbuf", bufs=1) as wbuf_pool,
            tc.tile_pool(name="ident", bufs=1) as id_pool,
            tc.tile_pool(name="xnat", bufs=3) as xnat_pool,
            tc.tile_pool(name="xtm", bufs=1) as xtm_pool,
            tc.tile_pool(name="otm", bufs=3) as otm_pool,
            tc.tile_pool(name="onat", bufs=3) as onat_pool,
            tc.tile_pool(name="ps_it", bufs=2, space="PSUM") as psit_pool,
            tc.tile_pool(name="ps_conv", bufs=3, space="PSUM") as psconv_pool,
            tc.tile_pool(name="ps_ot", bufs=2, space="PSUM") as psot_pool,
        ):
            wbuf = wbuf_pool.tile([128, nch, PACKW], bf16)
            biasbuf = wbuf_pool.tile([128, nch, 2], fp32, tag="biasbuf")
            ident = id_pool.tile([128, 128], bf16)
            nc.gpsimd.dma_start(ident[:], id_d)
            nc.gpsimd.dma_start(biasbuf[:], bias_v)
            for i in range(nch // WCH):
                nc.gpsimd.dma_start(wbuf[:, i * WCH:(i + 1) * WCH, :], w_v[i])

            # Two persistent xtm buffers (alternated by channel parity) so the
            # q9=0 zero columns are memset once, not per channel.
            xtm0 = xtm_pool.tile([128, 576], bf16, tag="xtm0", name="xtm0")
            xtm1 = xtm_pool.tile([128, 576], bf16, tag="xtm1", name="xtm1")
            xtms = [xtm0, xtm1]
            for xt in xtms:
                xvz = xt[:].rearrange("p (jj two b q9) -> p jj two b q9",
                                      jj=2, two=2, b=16, q9=9)
                nc.vector.memset(xvz[:, :, :, :, 0], 0.0)

            import contextlib
            loop_cm = (tc.For_i(0, loop_reps, 1) if loop_reps > 1
                       else contextlib.nullcontext())
            with loop_cm:
              for c in range(nch):
                  # --- load x natural [128=(b,q), 512t], transpose ---
                  # xtm col = j*144 + b*9 + q9; block k=4q+j of batch b at
                  # q9=q+1; q9=0 cols are zero (causal carry for each batch's
                  # first chunk, via the j=3 segment).
                  xnat = xnat_pool.tile([128, 512], bf16)
                  xtm = xtms[c % 2]
                  xv = xtm[:].rearrange("p (jj two b q9) -> p jj two b q9",
                                        jj=2, two=2, b=16, q9=9)
                  if "indma" not in skip:
                      nc.sync.dma_start(xnat[:], x_v[c])
                  ps_it = psit_pool.tile([128, 512], fp32, tag="ps_it")
                  if "intrans" not in skip:
                      for j in range(4):
                          nc.tensor.matmul(
                              ps_it[:, j * 128:(j + 1) * 128],
                              xnat[:, j * 128:(j + 1) * 128],
                              ident[:],
                              start=True, stop=True, skip_group_check=True)
                      if "inevac" not in skip:
                          nc.vector.tensor_copy(
                              xv[:, :, :, :, 1:9],
                              ps_it[:].rearrange("p (jj two b q) -> p jj two b q",
                                                 jj=2, two=2, b=16))

                  # --- conv matmuls (bf16) ---
                  # psum cols ordered (up, b, uh): u = 2*uh + up.
                  rhs_even = xv[:, :, 0, :, 1:9]      # block 2u   = (jj=up, two=0)
                  rhs_odd = xv[:, :, 1, :, 1:9]       # block 2u+1 = (jj=up, two=1)
                  wA_e = wbuf[:, c, 0:128]
                  wA_o = wbuf[:, c, 128:256]
                  wB = wbuf[0:32, c, 256:384]
                  wCe = wbuf[:, c, 256:288]
                  wCo = wbuf[:, c, 288:320]

                  ps_e = psconv_pool.tile([128, 256], fp32, tag="ps_conv")
                  ps_o = psconv_pool.tile([128, 256], fp32, tag="ps_conv")
                  if "conv" not in skip:
                    nc.tensor.matmul(ps_e[:], wA_e, rhs_even,
                                     start=True, stop=False, skip_group_check=True)
                    nc.tensor.matmul(ps_e[:], wB, xv[0:32, :, 1, :, 1:9],
                                     start=False, stop=False, skip_group_check=True)
                    # causal carry rhs, cols (up, b, uh):
                    # col = 432 - 287*up + 9b + uh; up=0 hits the j=3 segment
                    # (q9=0 -> zero col), up=1 the j=1 segment.
                    rhs_prev = BassAP(tensor=xtm[:].tensor, offset=432,
                                      ap=[[576, 128], [-287, 2], [9, 16], [1, 8]])
                    nc.tensor.matmul(ps_e[0:32, :], wCe, rhs_prev,
                                     start=False, stop=True, skip_group_check=True)

                    nc.tensor.matmul(ps_o[:], wA_o, rhs_odd,
                                     start=True, stop=False, skip_group_check=True)
                    nc.tensor.matmul(ps_o[0:32, :], wCo, rhs_even,
                                     start=False, stop=True, skip_group_check=True)

                  # --- evacuate with bias (ACT) ---
                  otm_e = otm_pool.tile([128, 256], bf16, tag="otm")
                  otm_o = otm_pool.tile([128, 256], bf16, tag="otm")
                  if "act" not in skip:
                      nc.scalar.activation(otm_e[:], ps_e[:], COPY,
                                           bias=biasbuf[:, c, 0:1])
                      nc.scalar.activation(otm_o[:], ps_o[:], COPY,
                                           bias=biasbuf[:, c, 1:2])

                  # --- transpose back to natural [128=(b,q), 512t], store ---
                  onat = onat_pool.tile([128, 512], bf16, tag="onat")
                  for half in range(2):
                      if "outtrans" in skip:
                          break
                      ps = psot_pool.tile([128, 256], fp32, tag="ps_ot")
                      for par, otm in ((0, otm_e), (1, otm_o)):
                          nc.tensor.matmul(
                              ps[:, par * 128:(par + 1) * 128],
                              otm[:, half * 128:(half + 1) * 128],
                              ident[:],
                              start=True, stop=True, skip_group_check=True)
                      if "outevac" not in skip:
                          if half == 0:
                              nc.vector.tensor_copy(
                                  onat[:, half * 256:(half + 1) * 256], ps[:])
                          else:
                              nc.scalar.activation(
                                  onat[:, half * 256:(half + 1) * 256], ps[:],
                                  mybir.ActivationFunctionType.Copy, bias=0.0)
                  if "outdma" not in skip:
                      nc.scalar.dma_start(o_v[c], onat[:])

    nc.compile()
    return nc


def make_core_inputs(x, causal_w, causal_b, chunk_w, chunk_b, conv_scale):
    """Shard host-side inputs for the 8 cores (bf16 conversion included)."""
    x = np.asarray(x, np.float32).astype(BF16)
    pack, bias = _pack_weights(causal_w, causal_b, chunk_w, chunk_b, conv_scale)
    ident = np.eye(128, dtype=BF16)
    in_maps = []
    for i in range(NCORES):
        sl = slice(i * NCH, (i + 1) * NCH)
        in_maps.append({
            "x": np.ascontiguousarray(x[:, sl, :]),
            "wpack": np.ascontiguousarray(pack[sl]),
            "bias": np.ascontiguousarray(bias[sl]),
            "ident": ident,
        })
    return in_maps


def kernel(x, causal_w, causal_b, chunk_w, chunk_b, conv_scale, chunk_size):
    from concourse.bass_utils import run_bass_kernel_spmd

    assert int(chunk_size) == 256
    in_maps = make_core_inputs(x, causal_w, causal_b, chunk_w, chunk_b,
                               conv_scale)
    nc = build_nc()
    core_ids = list(range(NCORES))
    res = run_bass_kernel_spmd(nc, in_maps, core_ids)
    out = np.empty((B, C, T), np.float32)
    for i in core_ids:
        out[:, i * NCH:(i + 1) * NCH, :] = res.results[i]["out"].astype(np.float32)
    return out
